# revision 56
# baseline (speedup 1.0000x reference)
"""Trainium2 Bass kernel for nn_HausdorffDistance_28406913696124.

Math (reference):
    px = (prob_map[0].ravel() >= 0.5)                 # [N], N = 100*100
    py = (gt_map.ravel()   >= 0.5)                    # [N]
    D[i,j] = euclid dist between grid points i, j     # [N, N] constant!
    loss   = mean_i | px_i * mean_j D[i,j] - (D @ py)_i / N |

Key structure: D depends only on (|r_i-r_j|, |c_i-c_j|), so

  * rowmean_i = mean_j D[i,j] is a pure constant -> precomputed on host.
  * (D @ py) is the 2D correlation of the binary mask PY with the radial
    kernel Q[u,v] = sqrt(u^2+v^2), u,v in [0,100).  A rank-R truncated
    (multiplicity-weighted) SVD  Q ~= sum_k a_k b_k^T  makes the
    correlation separable:

        term2sum = sum_k  A_k @ PY @ B_k,
        A_k[r,r'] = a_k[|r-r'|],  B_k[c,c'] = b_k[|c-c'|]   (sym Toeplitz)

    R = 2 already gives ~2e-5 relative error on the final scalar
    (tolerance 2e-2): the |.| + mean over 10^4 pixels buries both the
    truncation and the fp16 rounding noise.

On device this is two matmul stages on a SINGLE core (no collective):
    stage 1:  U_k[r',c] = sum_c' PYT[c',r'] * B_k[c',c]
              (one matmul per PSUM bank, stationary = binarized mask^T)
    stage 2:  R accumulating matmuls  term2 += A_k^T @ U_k.
Tail (sign decomposition, no abs needed since term2sum <= rowsumN):
    |diff| = (1-2*px)*term2sum + px*rowsumN; one fused
    scalar_tensor_tensor with accum_out gives the per-row sums, a pair
    of accumulating ones-matmuls the cross-partition total.  The scalar
    leaves through a PREPARED SWDGE scatter descriptor fired by
    trigger_dma (no HWDGE/DGE-delay on the critical path); the
    1/(N^2*SCALE) rescale happens host-side on the returned value.
"""

import sys

import numpy as np

sys.path.insert(0, "/opt/trn_rl_repo")

H = 100
N = H * H
R = 2          # SVD rank of the distance kernel
# Scale folded into Acat + rowsumN so no on-device final scaling is
# needed; 2^-13 is mantissa-exact in f16/f32.  The host multiplies the
# returned raw |diff| total by 1 / (N^2 * SCALE) when unpacking.
SCALE = 2.0 ** -13
# NOTE: an earlier variant seeded the stage-2 PSUM bank with the px*rowsum
# term from the DVE and let the matmuls accumulate onto it
# (skip_group_check) — that raced on real HW (~25% of first executions lost
# part of a matmul contribution).  The sign-decomposition tail below needs
# no cross-engine PSUM write at all.


def _host_constants():
    """Geometry-only constant tables (input independent)."""
    idx = np.arange(H)
    absdiff = np.abs(idx[:, None] - idx[None, :])  # [100,100] |b-c|
    q = np.sqrt((idx[:, None] ** 2 + idx[None, :] ** 2).astype(np.float64))

    # rowsum[r,c] = sum_j D[i,j] (i = r*100+c) in float64, negated so the
    # device can seed the PSUM accumulation with px * (-rowsumN), and
    # pre-scaled by SCALE (matching Acat) so no device-side scaling of
    # the final reduction is needed.
    cnt = np.zeros((H, H))  # cnt[r,u] = #{a : |r-a| = u}
    np.add.at(cnt, (idx[:, None], absdiff), 1.0)
    rowsumn = (-SCALE * (cnt @ q @ cnt.T)).astype(np.float32)

    # multiplicity-weighted rank-R SVD of Q (weights = how often each
    # (u,v) displacement occurs in the 100x100 grid)
    m = np.where(idx == 0, 100.0, 2.0 * (100 - idx))
    sw = np.sqrt(m)
    uu, ss, vt = np.linalg.svd(sw[:, None] * q * sw[None, :])
    a = (uu[:, :R] * np.sqrt(ss[:R])) / sw[:, None]
    b = (vt[:R, :].T * np.sqrt(ss[:R])) / sw[:, None]

    # Bcat[c', k*100+c] = b_k[|c'-c|];  Acat[r', k*100+r] = a_k[|r'-r|]
    # (Acat carries the SCALE factor; 2^-13 keeps f16 mantissas exact.)
    bcat = np.concatenate(
        [b[absdiff, k] for k in range(R)], axis=1).astype(np.float16)
    acat = np.concatenate(
        [SCALE * a[absdiff, k] for k in range(R)], axis=1).astype(np.float16)
    return rowsumn, bcat, acat


def _build_module():
    import concourse.bacc as bacc
    import concourse.mybir as mybir
    import concourse.tile as tile

    f32 = mybir.dt.float32
    f16 = mybir.dt.float16

    nc = bacc.Bacc(
        "TRN2",
        target_bir_lowering=False,
        debug=False,
        enable_asserts=False,
        num_devices=1,
    )

    # One packed input: Bcat | Acat | rowsumN_neg(f32 as 2xf16) | gtT-.5
    # | prob-.5   ([100, 2R*100 + 400] f16).  A single DMA instruction
    # pays the fixed HWDGE(625) + dge-delay(650) + sem-prop(900) once.
    PK = 2 * R * H + 4 * H
    pack_d = nc.dram_tensor("pack", [H, PK], f16, kind="ExternalInput")
    # dma_scatter_add payload granularity is 256B = 64 f32; the scalar
    # result lands in out[0, 0], the rest is junk the host ignores.
    out_d = nc.dram_tensor("out", [1, 64], f32, kind="ExternalOutput")

    with tile.TileContext(nc) as tc:
        with (
            tc.tile_pool(name="sb", bufs=1) as sb,
            tc.tile_pool(name="ps_u", bufs=1, space="PSUM") as ps_u,
            tc.tile_pool(name="ps_u2", bufs=1, space="PSUM") as ps_u2,
            tc.tile_pool(name="ps_t2", bufs=1, space="PSUM") as ps_t2,
            tc.tile_pool(name="ps_fin", bufs=1, space="PSUM") as ps_fin,
        ):
            pack_sb = sb.tile([H, PK], f16)
            nc.sync.dma_start(pack_sb[:], pack_d[:])

            # ---- output path setup, all off the critical path -----------
            # A single-descriptor dma_scatter_add (index 0) ships
            # pay[0, 0:64] to the output.  Descriptors are PREPARED early
            # on the idle Pool engine so firing them later only costs a
            # trigger + the completion-sem latency (saves the 625ns HWDGE
            # + 650ns DGE delay of a plain store).  The runner pre-zeros
            # ExternalOutput buffers, so the += lands on zeros.
            from concourse import library_config
            nc.gpsimd.load_library(library_config.mlp)
            idx_sb = sb.tile([128, 1], mybir.dt.int16)
            nc.vector.memset(idx_sb[:], 0)
            pay = sb.tile([128, 64], f32)
            nc.vector.memset(pay[:], 0.0)
            out_sem = nc.alloc_semaphore("out_dma")
            nc.gpsimd.dma_scatter_add(
                out_d[:],
                pay[:].rearrange("p (a b) -> p a b", a=1),
                idx_sb[:],
                1,
                1,
                64,
                prepare_only=True,
                sem=out_sem,
            )
            bcat_sb = pack_sb[:, 0:R * H]
            acat_sb = pack_sb[:, R * H:2 * R * H]
            rsn_sb = pack_sb[:, 2 * R * H:2 * R * H + 2 * H].bitcast(f32)
            gtt_sb = pack_sb[:, 2 * R * H + 2 * H:2 * R * H + 3 * H]
            prob_sb = pack_sb[:, 2 * R * H + 3 * H:PK]

            # ---- binarize the transposed mask (f16 0/1) -----------------
            pyt = sb.tile([H, H], f16)
            nc.vector.tensor_scalar(
                pyt[:], gtt_sb, 0.0, None, mybir.AluOpType.is_ge
            )

            # ---- stage 1: U_k = PYT^T @ B_k, one matmul per PSUM bank so
            #      the ACT and DVE engines can downcast the two halves to
            #      SBUF in parallel (same-bank readers serialize) --------
            u0_ps = ps_u.tile([H, H], f32)
            u1_ps = ps_u2.tile([H, H], f32)
            nc.tensor.matmul(
                u0_ps[:], pyt[:], bcat_sb[:, 0:H], start=True, stop=True)
            nc.tensor.matmul(
                u1_ps[:], pyt[:], bcat_sb[:, H:R * H], start=True, stop=True)
            u0_sb = sb.tile([H, H], f16)
            u1_sb = sb.tile([H, H], f16)
            nc.vector.tensor_copy(u0_sb[:], u0_ps[:])
            nc.scalar.activation(
                u1_sb[:], u1_ps[:], mybir.ActivationFunctionType.Copy
            )

            # ---- sign decomposition of the |.|: with s = 1-2*px,
            #      |diff| = s*term2sum + px*rowsumN elementwise (term2sum
            #      <= rowsumN always, py being a 0/1 mask), so no abs is
            #      needed and the row reduction fuses into the s*t2
            #      multiply via scalar_tensor_tensor's accum_out.  The
            #      px*rowsumN row sums enter through a second accumulating
            #      ones-matmul (with a -1 vector: t1p carries -rowsumN).
            #      Everything here is off the critical path. --------------
            t2_ps = ps_t2.tile([H, H], f32)
            px_sb = sb.tile([H, H], f16)
            nc.vector.tensor_scalar(
                px_sb[:], prob_sb, 0.0, None, mybir.AluOpType.is_ge
            )
            s_sb = sb.tile([H, H], f16)
            nc.vector.tensor_scalar(
                s_sb[:], px_sb[:], -2.0, 1.0,
                mybir.AluOpType.mult, mybir.AluOpType.add,
            )
            t1p_sb = sb.tile([H, H], f32)
            nc.vector.scalar_tensor_tensor(
                t1p_sb[:],
                prob_sb,
                0.0,
                rsn_sb,
                op0=mybir.AluOpType.is_ge,
                op1=mybir.AluOpType.mult,
            )
            rowbneg = sb.tile([H, 1], f32)
            nc.vector.tensor_reduce(
                rowbneg[:],
                t1p_sb[:],
                axis=mybir.AxisListType.X,
                op=mybir.AluOpType.add,
            )


            # ---- stage 2: diff = t1n + sum_k A_k^T @ U_k ----------------
            for k, u_sb in enumerate([u0_sb, u1_sb]):
                nc.tensor.matmul(
                    t2_ps[:],
                    acat_sb[:, k * H:(k + 1) * H],
                    u_sb[:],
                    start=(k == 0),
                    stop=(k == R - 1),
                )

            # ---- tail: ONE fused op gives the per-row |diff| sums:
            #      rowabs = rowB + sum_c s * t2; then cross-partition sum
            #      via a ones-matmul (PE), copy the scalar into the
            #      scatter payload's partition 0, fire the prepared DMA.
            #      (Pool only ever runs [lib-load, prep, trigger], so the
            #      early prep is never stuck behind late Pool compute.) --
            ones_sb = sb.tile([H, 1], f32)
            nc.vector.memset(ones_sb[:], 1.0)
            onesn_sb = sb.tile([H, 1], f32)
            nc.vector.memset(onesn_sb[:], -1.0)
            fin_ps = ps_fin.tile([1, 1], f32)
            # rowB contribution: sum_i (-1) * rowbneg_i = +sum px*rowsumN
            nc.tensor.matmul(
                fin_ps[:], rowbneg[:], onesn_sb[:], start=True, stop=False
            )
            # critical tail: rowA = per-row sums of s*t2 in ONE fused op
            scr_sb = sb.tile([H, H], f32)
            rowa = sb.tile([H, 1], f32)
            nc.vector.scalar_tensor_tensor(
                scr_sb[:],
                t2_ps[:],
                0.0,
                s_sb[:],
                op0=mybir.AluOpType.bypass,
                op1=mybir.AluOpType.mult,
                accum_out=rowa[:],
            )
            nc.tensor.matmul(
                fin_ps[:], rowa[:], ones_sb[:], start=False, stop=True
            )
            nc.vector.tensor_copy(pay[0:1, 0:1], fin_ps[:])
            nc.gpsimd.trigger_dma(count=None)

    # Tile's gen_mode==1 sem plumbing gap: the epilogue gate waits on the
    # DMASW lane semaphore, but a PREPARED descriptor bumps the caller's
    # sem= semaphore instead (routed to on_update[0] at prep time).
    # Retarget the wait at the semaphore the descriptor actually updates.
    dma_upd = None
    for blk in nc.m.functions[0].blocks:
        for inst in blk.instructions:
            if type(inst).__name__ == "InstDMAScatterAddAnt":
                dma_upd = inst.sync_info.on_update[0]
    assert dma_upd is not None and dma_upd.ant_name == "out_dma"
    for blk in nc.m.functions[0].blocks:
        for inst in blk.instructions:
            si = getattr(inst, "sync_info", None)
            if si is None:
                continue
            for w in si.on_wait:
                if str(getattr(w, "ant_name", "")).startswith("DMASW"):
                    w.id = dma_upd.id
                    w.ant_name = dma_upd.ant_name

    nc.compile()
    return nc


_STATE = {}


def _get_state():
    if not _STATE:
        rowsumn, bcat, acat = _host_constants()
        pack = np.empty((H, 2 * R * H + 4 * H), dtype=np.float16)
        pack[:, 0:R * H] = bcat
        pack[:, R * H:2 * R * H] = acat
        pack[:, 2 * R * H:2 * R * H + 2 * H] = rowsumn.view(np.float16)
        _STATE["pack"] = pack
        _STATE["nc"] = _build_module()
    return _STATE


def _in_maps(prob_map, gt_map):
    st = _get_state()
    pack = st["pack"]
    prob = np.asarray(prob_map, dtype=np.float32).reshape(H, H)
    gt = np.asarray(gt_map, dtype=np.float32).reshape(H, H)
    # x - 0.5 is sign-exact in f32; the f16 cast can only flip the
    # comparison for |x-0.5| < 2^-25 (measure ~3e-8 per element).
    pack[:, 2 * R * H + 2 * H:2 * R * H + 3 * H] = (
        gt.T - np.float32(0.5)).astype(np.float16)
    pack[:, 2 * R * H + 3 * H:] = (prob - np.float32(0.5)).astype(np.float16)
    return [{"pack": np.ascontiguousarray(pack)}]


def _run(prob_map, gt_map, trace=False, **spmd_kwargs):
    from concourse import bass_utils

    st = _get_state()
    in_maps = _in_maps(prob_map, gt_map)
    res = bass_utils.run_bass_kernel_spmd(
        st["nc"], in_maps, core_ids=[0], trace=trace, **spmd_kwargs,
    )
    raw = np.float64(res.results[0]["out"][0, 0])
    value = np.float32(raw / (SCALE * N * N))
    return value, res


def kernel(prob_map, gt_map):
    value, _ = _run(prob_map, gt_map, trace=False)
    return np.asarray(value, dtype=np.float32)
